# revision 5
# baseline (speedup 1.0000x reference)
"""ConVIRT loss kernel for 8 Trainium2 NeuronCores.

Computation (reference):
    vn = v / max(||v||, eps);  un = u / max(||u||, eps)          [8192, 768]
    sim = vn @ un.T / TAU                                        [8192, 8192]
    loss_it = logsumexp(sim, axis=1) - diag(sim)
    loss_ti = logsumexp(sim, axis=0) - diag(sim)
    out = mean(0.75 * loss_it + 0.25 * loss_ti)                  scalar

Sharding: rows of v are split across the 8 cores (1024 rows each); every
core holds all of u.  Core c computes its [1024, 8192] slab of
exp(sim / TAU) on the fly and reduces it two ways (row sums and column
sums); the host takes logs, adds the exact diagonal, and averages.

Per-core pipeline (v2), all in fp8:
  - PE: S = vT.T @ uT in fp8 DoubleRow ([128,512] tiles, K=768 via 3
    instructions), plus column sums via an fp8 DoubleRow ones-matmul on
    E pairs (ones[128,2,32] @ E[128,2,1024] -> [32,1024] PSUM,
    accumulated over the 4 m-pairs of each 1024-column block).
  - ACT (~2/3 of tiles): E = exp(S * es) -> fp8e4 SBUF with the row sum
    taken for free via the fp32 accumulator (accum_out).
  - DVE (~1/3 of tiles): fast-exp via exponent-bit arithmetic — the
    fp8e4 BITS of exp(x) are an affine function of x per binade:
    i8 = rint(A8*S + B8) computed by one tensor_scalar (f32 PSUM in,
    int8 out reinterpreted as fp8e4), with B8 calibrated so the
    piecewise-linear approximation is mean-unbiased (rel err ~3% RMS
    per element, ~0.1% per 1024-sum; loss impact ~1e-4).  A second
    in-place tensor_scalar supplies the row sum via accum_out.
    DVE also copies the per-block column sums PSUM -> SBUF.
Row/column sums use sums of the same fp8-quantized E on the colsum path
for both engines, so the two paths mix freely.  The host normalizes /
casts / transposes inputs, computes the exact diagonal, and takes logs.
No max-subtraction is needed: |logits| <= 1/TAU = 10.

Device layout per core:
  vT  [768, 1024] fp8  (normalized v slab * 32, feature-major)
  uT  [768, 8192] fp8  (normalized u * 32, feature-major)
  rs  [128, 8]    f32  row sums of exp:  row m*128+p -> rs[p, m]
  cs  [1, 8192]   f32  column sums over the 1024 local rows
"""

import sys

for _p in ("/opt/trn_rl_repo", "/root/.axon_site/_ro/trn_rl_repo"):
    if _p not in sys.path:
        sys.path.insert(0, _p)

import numpy as np
import ml_dtypes

TAU = 0.1
LAMBD = 0.75
EPS = 1e-8
B, D = 8192, 768
N_CORES = 8
M_ROWS = B // N_CORES          # 1024 rows of v per core
M_TILES = M_ROWS // 128        # 8
K_TILES = D // 128             # 6
NB = 8                         # column blocks of 1024
NB_W = B // NB                 # 1024 columns per block
FP8_SCALE = 32.0               # host pre-scale before e4m3 cast
ES = 1.0 / (TAU * FP8_SCALE * FP8_SCALE)   # exp arg = S * ES
# fast-exp constants: fp8e4 bits of exp(S*ES) ~= rint(A8*S + B8)
A8 = 8.0 * np.log2(np.e) * ES
B8 = 56.0 - 0.4560             # calibrated: mean-unbiased vs exact exp

_CACHE = {}


def _is_dve_unit(u, dve_mod=3, dve_off=1):
    return (u % dve_mod) == dve_off


def build_nc(repeat=1, for_sim=False, dtype_mode="fp8", dve_mod=3, dve_off=1,
             upool_bufs=2, epool_bufs=3, spool_bufs=3, hoist_ut=False):
    """Per-core Bass module. `repeat` unrolls the pass for steady-state
    timing (outputs overwritten each repetition)."""
    import concourse.mybir as mybir
    import concourse.tile as tile
    from concourse import bacc

    f32 = mybir.dt.float32
    i8 = mybir.dt.int8
    fp8 = mybir.dt.float8e4
    DR = mybir.MatmulPerfMode.DoubleRow

    nc = bacc.Bacc("TRN2", target_bir_lowering=False)
    vT = nc.dram_tensor("vT", [D, M_ROWS], fp8, kind="ExternalInput")
    uT = nc.dram_tensor("uT", [D, B], fp8, kind="ExternalInput")
    rs_d = nc.dram_tensor("rs", [128, M_TILES], f32, kind="ExternalOutput")
    cs_d = nc.dram_tensor("cs", [1, B], f32, kind="ExternalOutput")

    with tile.TileContext(nc) as tc:
        with (
            tc.tile_pool(name="singles", bufs=1) as singles,
            tc.tile_pool(name="boundary", bufs=2) as boundary,
            tc.tile_pool(name="upool", bufs=upool_bufs) as upool,
            tc.tile_pool(name="epool", bufs=epool_bufs) as epool,
            tc.tile_pool(name="spool", bufs=spool_bufs, space="PSUM") as spool,
            tc.tile_pool(name="cspool", bufs=1, space="PSUM") as cspool,
        ):
            ones = singles.tile([128, 2, 32], fp8)
            nc.vector.memset(ones, 1.0)
            # Preload the exp table set while DMAs run.
            dummy = singles.tile([128, 1], f32)
            nc.vector.memset(dummy, 0.0)
            nc.scalar.activation(out=dummy, in_=dummy,
                                 func=mybir.ActivationFunctionType.Exp)

            vT_sb = singles.tile([128, K_TILES, M_ROWS], fp8)
            nc.sync.dma_start(
                out=vT_sb[:, :, :],
                in_=vT.rearrange("(k p) b -> p k b", p=128))

            uT_r = uT.rearrange("(k p) b -> p k b", p=128)
            if hoist_ut:
                uT_hoisted = singles.tile([128, K_TILES, B], fp8)
                for nb in range(NB):
                    nc.sync.dma_start(
                        out=uT_hoisted[:, :, nb * NB_W:(nb + 1) * NB_W],
                        in_=uT_r[:, :, nb * NB_W:(nb + 1) * NB_W])

            for rep in range(repeat):
                rs_parts = boundary.tile([128, M_TILES, NB], f32,
                                         tag="rs_parts")
                colsum_sb = boundary.tile([1, B], f32, tag="colsum_sb")

                if hoist_ut:
                    uT_sb = uT_hoisted
                else:
                    uT_sb = upool.tile([128, K_TILES, B], fp8, tag="uT")
                    for nb in range(NB):
                        nc.sync.dma_start(
                            out=uT_sb[:, :, nb * NB_W:(nb + 1) * NB_W],
                            in_=uT_r[:, :, nb * NB_W:(nb + 1) * NB_W])

                pending_cs = []
                for nb in range(NB):
                    cs_ps = cspool.tile([32, NB_W], f32, tag="cs")
                    for m in range(M_TILES):
                        if m % 2 == 0:
                            ep = epool.tile([128, 2, NB_W], fp8, tag="E")
                        s = spool.tile([128, NB_W], f32, tag="S")
                        for kp in range(K_TILES // 2):
                            lhsT = vT_sb[:, 2 * kp:2 * kp + 2,
                                         m * 128:(m + 1) * 128]
                            for ns in range(NB_W // 512):
                                nc.tensor.matmul(
                                    s[:, ns * 512:(ns + 1) * 512],
                                    lhsT,
                                    uT_sb[:, 2 * kp:2 * kp + 2,
                                          nb * NB_W + ns * 512:
                                          nb * NB_W + (ns + 1) * 512],
                                    start=(kp == 0),
                                    stop=(kp == K_TILES // 2 - 1),
                                    perf_mode=DR,
                                )
                        # flush a delayed colsum matmul so PE never waits
                        # on the exp of the pair it reduces
                        for fn in pending_cs:
                            fn()
                        pending_cs = []
                        eh = ep[:, m % 2, :]
                        if _is_dve_unit(nb * M_TILES + m, dve_mod, dve_off):
                            nc.vector.tensor_scalar(
                                out=eh.bitcast(i8), in0=s,
                                scalar1=A8, scalar2=B8,
                                op0=mybir.AluOpType.mult,
                                op1=mybir.AluOpType.add)
                            nc.vector.tensor_scalar(
                                out=eh, in0=eh,
                                scalar1=1.0, scalar2=None,
                                op0=mybir.AluOpType.mult,
                                op1=mybir.AluOpType.add,
                                accum_out=rs_parts[:, m, nb:nb + 1])
                        else:
                            nc.scalar.activation(
                                out=eh, in_=s,
                                func=mybir.ActivationFunctionType.Exp,
                                scale=ES,
                                accum_out=rs_parts[:, m, nb:nb + 1])
                        if m % 2 == 1:
                            def make_cs(ep=ep, cs_ps=cs_ps, m=m):
                                def emit():
                                    for h in range(NB_W // 512):
                                        nc.tensor.matmul(
                                            cs_ps[:, h * 512:(h + 1) * 512],
                                            ones,
                                            ep[:, :, h * 512:(h + 1) * 512],
                                            start=(m == 1),
                                            stop=(m == M_TILES - 1),
                                            perf_mode=DR)
                                return emit
                            pending_cs.append(make_cs())
                    # copy this block's colsums out (row 0 of cs_ps).
                    # The last pair's matmul is still pending; emit the copy
                    # after it via a deferred closure too.
                    def make_copy(cs_ps=cs_ps, nb=nb):
                        def emit():
                            nc.vector.tensor_copy(
                                out=colsum_sb[0:1, nb * NB_W:(nb + 1) * NB_W],
                                in_=cs_ps[0:1, :])
                        return emit
                    pending_cs.append(make_copy())
                for fn in pending_cs:
                    fn()

                rs_fin = boundary.tile([128, M_TILES, 1], f32, tag="rs_fin")
                nc.vector.reduce_sum(out=rs_fin, in_=rs_parts,
                                     axis=mybir.AxisListType.X)
                nc.sync.dma_start(out=rs_d[:, :], in_=rs_fin[:, :, 0])
                nc.sync.dma_start(out=cs_d[:, :], in_=colsum_sb[:, :])

    if for_sim:
        nc.compile()
    else:
        nc.finalize()
    return nc


def prep_inputs(v, u, dtype_mode="fp8"):
    """Host-side prep: normalize rows, scale+cast to fp8e4, transpose to
    feature-major, shard v across cores. Returns (in_maps, vn, un)."""
    v = np.asarray(v, dtype=np.float32)
    u = np.asarray(u, dtype=np.float32)
    vn = v / np.maximum(np.sqrt((v.astype(np.float64) ** 2).sum(1)),
                        EPS).astype(np.float32)[:, None]
    un = u / np.maximum(np.sqrt((u.astype(np.float64) ** 2).sum(1)),
                        EPS).astype(np.float32)[:, None]
    dt = ml_dtypes.float8_e4m3
    vnT = np.ascontiguousarray((vn.T * FP8_SCALE).astype(dt))
    unT = np.ascontiguousarray((un.T * FP8_SCALE).astype(dt))
    in_maps = [
        {"vT": np.ascontiguousarray(vnT[:, c * M_ROWS:(c + 1) * M_ROWS]),
         "uT": unT}
        for c in range(N_CORES)
    ]
    return in_maps, vn, un


def combine(results, vn, un):
    """Host-side unshard: logs + exact diagonal + weighted mean."""
    rowsum = np.concatenate(
        [np.asarray(r["rs"], np.float64).T.reshape(-1) for r in results])
    colsum = np.sum(
        [np.asarray(r["cs"], np.float64)[0] for r in results], axis=0)
    diag = (vn.astype(np.float64) * un.astype(np.float64)).sum(1) / TAU
    lse_r = np.log(rowsum)
    lse_c = np.log(colsum)
    loss = np.mean(LAMBD * (lse_r - diag) + (1.0 - LAMBD) * (lse_c - diag))
    return np.asarray(loss, dtype=np.float32)


DTYPE_MODE = "fp8"


def kernel(v, u):
    from concourse.bass_utils import run_bass_kernel_spmd

    if "nc" not in _CACHE:
        _CACHE["nc"] = build_nc(dtype_mode=DTYPE_MODE)
    nc = _CACHE["nc"]
    in_maps, vn, un = prep_inputs(v, u, dtype_mode=DTYPE_MODE)
    res = run_bass_kernel_spmd(nc, in_maps, core_ids=list(range(N_CORES)))
    return combine(res.results, vn, un)


if __name__ == "__main__":
    rng = np.random.default_rng(0)
    v = rng.standard_normal((B, D), dtype=np.float32)
    u = rng.standard_normal((B, D), dtype=np.float32)
    out = kernel(v, u)
    print("kernel out:", out)


# revision 12
# speedup vs baseline: 1.2057x; 1.2057x over previous
"""ConVIRT loss kernel for 8 Trainium2 NeuronCores.

Computation (reference):
    vn = v / max(||v||, eps);  un = u / max(||u||, eps)          [8192, 768]
    sim = vn @ un.T / TAU                                        [8192, 8192]
    loss_it = logsumexp(sim, axis=1) - diag(sim)
    loss_ti = logsumexp(sim, axis=0) - diag(sim)
    out = mean(0.75 * loss_it + 0.25 * loss_ti)                  scalar

Sharding: rows of v are split across the 8 cores (1024 rows each); every
core holds all of u.  Core c computes its [1024, 8192] slab of
exp(sim / TAU) on the fly and reduces it two ways (row sums and column
sums); the host takes logs, adds the exact diagonal, and averages.

Per-core pipeline (v3).  On this part the dominant cost is the PE
instruction stream (~45 ns sequencer overhead per instruction on top of
the fp8 DoubleRow array time), so the design keeps PE to the bare
minimum — the 768 mandatory main-matmul instructions — and spreads
everything else across the other engines:
  - PE:   S = vT.T @ uT in fp8 DoubleRow, [128,512] PSUM tiles, K=768
          via 3 DoubleRow instructions.  Nothing else.
  - ACT:  E = exp(S*ES) -> bf16 SBUF, 2048 columns per instruction,
          row sums for free via the fp32 accumulator (accum_out).
  - DVE:  (optional, dve units) fast-exp via exponent-bit arithmetic:
          the bf16 BITS of exp(x) are an affine function of x per
          binade: i16 = rint(A16*S + B16) via one tensor_scalar, with
          B16 calibrated mean-unbiased (~0.04% error per 2048-sum);
          a second in-place tensor_scalar provides the row sum.
          DVE also accumulates column sums: e_acc += E per m-tile.
  - Pool: per-block column-sum finish via partition_all_reduce on
          e_acc (the only engine-partition reduction off the PE).
The host normalizes / scales / casts inputs to fp8e4, computes the
exact diagonal, and takes logs.  No max-subtraction is needed:
|logits| <= 1/TAU = 10.

Device layout per core:
  vT  [768, 1024] fp8  (normalized v slab * 32, feature-major)
  uT  [768, 8192] fp8  (normalized u * 32, feature-major)
  rs  [128, 8]    f32  row sums of exp:  row m*128+p -> rs[p, m]
  cs  [1, 8192]   f32  column sums over the 1024 local rows
"""

import sys

for _p in ("/opt/trn_rl_repo", "/root/.axon_site/_ro/trn_rl_repo"):
    if _p not in sys.path:
        sys.path.insert(0, _p)

import numpy as np
import ml_dtypes

TAU = 0.1
LAMBD = 0.75
EPS = 1e-8
B, D = 8192, 768
N_CORES = 8
M_ROWS = B // N_CORES          # 1024 rows of v per core
M_TILES = M_ROWS // 128        # 8
K_TILES = D // 128             # 6
NB = 4                         # column blocks of 2048
NB_W = B // NB                 # 2048 columns per block
FP8_SCALE = 32.0               # host pre-scale before e4m3 cast
ES = 1.0 / (TAU * FP8_SCALE * FP8_SCALE)   # exp arg = S * ES
# fast-exp constants: bf16 bits of exp(S*ES) ~= rint(A16*S + B16)
A16 = 128.0 * np.log2(np.e) * ES
B16 = 127.0 * 128.0 - 7.351    # calibrated: mean-unbiased vs exact exp

_CACHE = {}


def _is_dve_unit(u, dve_mod=8, dve_off=3):
    return (u % dve_mod) == dve_off


def build_nc(repeat=1, for_sim=False, dtype_mode="fp8", dve_mod=8, dve_off=3,
             upool_bufs=2, epool_bufs=3, spool_bufs=2, hoist_ut=False,
             ablate=(), mm_n=512):
    """Per-core Bass module. `repeat` unrolls the pass for steady-state
    timing (outputs overwritten each repetition)."""
    import concourse.mybir as mybir
    import concourse.tile as tile
    from concourse import bacc
    from concourse import bass_isa

    f32 = mybir.dt.float32
    i16 = mybir.dt.int16
    bf16 = mybir.dt.bfloat16
    fp8 = mybir.dt.float8e4
    DR = mybir.MatmulPerfMode.DoubleRow

    nc = bacc.Bacc("TRN2", target_bir_lowering=False)
    vT = nc.dram_tensor("vT", [D, M_ROWS], fp8, kind="ExternalInput")
    uT = nc.dram_tensor("uT", [D, B], fp8, kind="ExternalInput")
    rs_d = nc.dram_tensor("rs", [128, M_TILES], f32, kind="ExternalOutput")
    cs_d = nc.dram_tensor("cs", [1, B], f32, kind="ExternalOutput")

    with tile.TileContext(nc) as tc:
        with (
            tc.tile_pool(name="singles", bufs=1) as singles,
            tc.tile_pool(name="boundary", bufs=2) as boundary,
            tc.tile_pool(name="upool", bufs=upool_bufs) as upool,
            tc.tile_pool(name="epool", bufs=epool_bufs) as epool,
            tc.tile_pool(name="eaccpool", bufs=2) as eaccpool,
            tc.tile_pool(name="arpool", bufs=2) as arpool,
            tc.tile_pool(name="spool", bufs=spool_bufs, space="PSUM") as spool,
        ):
            # Preload the exp table set while DMAs run.
            dummy = singles.tile([128, 1], f32)
            nc.vector.memset(dummy, 0.0)
            nc.scalar.activation(out=dummy, in_=dummy,
                                 func=mybir.ActivationFunctionType.Exp)

            vT_sb = singles.tile([128, K_TILES, M_ROWS], fp8)
            nc.sync.dma_start(
                out=vT_sb[:, :, :],
                in_=vT.rearrange("(k p) b -> p k b", p=128))

            uT_r = uT.rearrange("(k p) b -> p k b", p=128)
            if hoist_ut:
                uT_hoisted = singles.tile([128, K_TILES, B], fp8)
                for nb in range(NB):
                    nc.sync.dma_start(
                        out=uT_hoisted[:, :, nb * NB_W:(nb + 1) * NB_W],
                        in_=uT_r[:, :, nb * NB_W:(nb + 1) * NB_W])

            for rep in range(repeat):
                rs_parts = boundary.tile([128, M_TILES, NB], f32,
                                         tag="rs_parts")
                if hoist_ut:
                    uT_sb = uT_hoisted
                else:
                    uT_sb = upool.tile([128, K_TILES, B], fp8, tag="uT")
                    for nb in range(NB):
                        nc.sync.dma_start(
                            out=uT_sb[:, :, nb * NB_W:(nb + 1) * NB_W],
                            in_=uT_r[:, :, nb * NB_W:(nb + 1) * NB_W])

                for nb in range(NB):
                    e_acc = eaccpool.tile([128, NB_W], bf16, tag="EA")
                    for m in range(M_TILES):
                        s = spool.tile([128, NB_W], f32, tag="S")
                        for kp in range(K_TILES // 2):
                            lhsT = vT_sb[:, 2 * kp:2 * kp + 2,
                                         m * 128:(m + 1) * 128]
                            for ns in range(NB_W // mm_n):
                                nc.tensor.matmul(
                                    s[:, ns * mm_n:(ns + 1) * mm_n],
                                    lhsT,
                                    uT_sb[:, 2 * kp:2 * kp + 2,
                                          nb * NB_W + ns * mm_n:
                                          nb * NB_W + (ns + 1) * mm_n],
                                    start=(kp == 0),
                                    stop=(kp == K_TILES // 2 - 1),
                                    perf_mode=DR,
                                )
                        if "noexp" in ablate:
                            if nb == 0 and m == 0:
                                nc.vector.tensor_copy(
                                    out=rs_parts[:, :, :],
                                    in_=s[:, 0:M_TILES * NB].rearrange(
                                        "p (a b) -> p a b", a=M_TILES))
                            continue
                        E = epool.tile([128, NB_W], bf16, tag="E")
                        if _is_dve_unit(nb * M_TILES + m, dve_mod, dve_off):
                            nc.vector.tensor_scalar(
                                out=E.bitcast(i16), in0=s,
                                scalar1=A16, scalar2=B16,
                                op0=mybir.AluOpType.mult,
                                op1=mybir.AluOpType.add)
                            nc.vector.tensor_scalar(
                                out=E, in0=E,
                                scalar1=1.0, scalar2=None,
                                op0=mybir.AluOpType.mult,
                                op1=mybir.AluOpType.add,
                                accum_out=rs_parts[:, m, nb:nb + 1])
                        else:
                            nc.scalar.activation(
                                out=E, in_=s,
                                func=mybir.ActivationFunctionType.Exp,
                                scale=ES,
                                accum_out=rs_parts[:, m, nb:nb + 1])
                        if m == 0:
                            nc.vector.tensor_copy(out=e_acc, in_=E)
                        else:
                            nc.vector.tensor_add(out=e_acc, in0=e_acc, in1=E)
                    if "noexp" in ablate:
                        continue
                    allred = arpool.tile([128, NB_W], f32, tag="AR")
                    nc.gpsimd.partition_all_reduce(
                        allred, e_acc, 128, bass_isa.ReduceOp.add)
                    nc.sync.dma_start(
                        out=cs_d[0:1, nb * NB_W:(nb + 1) * NB_W],
                        in_=allred[0:1, :])

                if "noexp" in ablate:
                    colsum_dummy = boundary.tile([1, B], f32, tag="cs0")
                    nc.vector.memset(colsum_dummy, 1.0)
                    nc.sync.dma_start(out=cs_d[:, :], in_=colsum_dummy)

                rs_fin = boundary.tile([128, M_TILES, 1], f32, tag="rs_fin")
                nc.vector.reduce_sum(out=rs_fin, in_=rs_parts,
                                     axis=mybir.AxisListType.X)
                nc.sync.dma_start(out=rs_d[:, :], in_=rs_fin[:, :, 0])

    if for_sim:
        nc.compile()
    else:
        nc.finalize()
    return nc


def prep_inputs(v, u, dtype_mode="fp8"):
    """Host-side prep: normalize rows, scale+cast to fp8e4, transpose to
    feature-major, shard v across cores. Returns (in_maps, vn, un)."""
    v = np.asarray(v, dtype=np.float32)
    u = np.asarray(u, dtype=np.float32)
    vn = v / np.maximum(np.sqrt((v.astype(np.float64) ** 2).sum(1)),
                        EPS).astype(np.float32)[:, None]
    un = u / np.maximum(np.sqrt((u.astype(np.float64) ** 2).sum(1)),
                        EPS).astype(np.float32)[:, None]
    dt = ml_dtypes.float8_e4m3
    vnT = np.ascontiguousarray((vn.T * FP8_SCALE).astype(dt))
    unT = np.ascontiguousarray((un.T * FP8_SCALE).astype(dt))
    in_maps = [
        {"vT": np.ascontiguousarray(vnT[:, c * M_ROWS:(c + 1) * M_ROWS]),
         "uT": unT}
        for c in range(N_CORES)
    ]
    return in_maps, vn, un


def combine(results, vn, un):
    """Host-side unshard: logs + exact diagonal + weighted mean."""
    rowsum = np.concatenate(
        [np.asarray(r["rs"], np.float64).T.reshape(-1) for r in results])
    colsum = np.sum(
        [np.asarray(r["cs"], np.float64)[0] for r in results], axis=0)
    diag = (vn.astype(np.float64) * un.astype(np.float64)).sum(1) / TAU
    lse_r = np.log(rowsum)
    lse_c = np.log(colsum)
    loss = np.mean(LAMBD * (lse_r - diag) + (1.0 - LAMBD) * (lse_c - diag))
    return np.asarray(loss, dtype=np.float32)


DTYPE_MODE = "fp8"


def kernel(v, u):
    from concourse.bass_utils import run_bass_kernel_spmd

    if "nc" not in _CACHE:
        _CACHE["nc"] = build_nc(dtype_mode=DTYPE_MODE)
    nc = _CACHE["nc"]
    in_maps, vn, un = prep_inputs(v, u, dtype_mode=DTYPE_MODE)
    res = run_bass_kernel_spmd(nc, in_maps, core_ids=list(range(N_CORES)))
    return combine(res.results, vn, un)


if __name__ == "__main__":
    rng = np.random.default_rng(0)
    v = rng.standard_normal((B, D), dtype=np.float32)
    u = rng.standard_normal((B, D), dtype=np.float32)
    out = kernel(v, u)
    print("kernel out:", out)
